# revision 33
# baseline (speedup 1.0000x reference)
"""Multi-head attention (B=2, S=2048, H=1024, 16 heads, RoPE) on 8 trn2 cores.

Sharding: core = (batch b, head-group g); b = core // 4, g = core % 4.
Each core computes 4 heads' attention for one batch and a partial output
projection; the host sums the 4 partials per batch.

All matmuls run as float32r (full-rate fp32 PE mode, ~1.5e-4 rel err).
Attention scores are computed directly in transposed [k, q] layout so the
attn @ V contraction needs no transposes; softmax normalization is deferred:
V carries an extra ones-column so the attention matmul also produces the
softmax denominator, and gpsimd partition_broadcast replicates 1/rowsum
across partitions for the final scale.
"""
import sys

import numpy as np

sys.path.insert(0, "/opt/trn_rl_repo")

import concourse.bass as bass  # noqa: E402
import concourse.mybir as mybir  # noqa: E402
import concourse.tile as tile  # noqa: E402
from concourse import bacc  # noqa: E402
from concourse.bass_utils import run_bass_kernel_spmd  # noqa: E402

F32 = mybir.dt.float32
F32R = mybir.dt.float32r
EXP = mybir.ActivationFunctionType.Exp

B, S, H = 2, 2048, 1024
NH, D = 16, 64                  # heads, head dim
GH = 4                          # heads per core (group)
GD = GH * D                     # 256 out dims per core
KT = H // 128                   # 8 contraction tiles for projections
MC = S // 128                   # 16 seq chunks of 128
QB = S // 512                   # 4 query blocks of 512
ROPE_BASE = 10000.0
SCALE = D ** -0.5


def _rope_tables():
    inv_freq = 1.0 / (ROPE_BASE ** (np.arange(0, D, 2, dtype=np.float64) / D))
    t = np.arange(S, dtype=np.float64)
    freqs = np.outer(t, inv_freq)                     # (S, 32)
    emb = np.concatenate([freqs, freqs], axis=-1)     # (S, 64)
    cosT = np.cos(emb).T.astype(np.float32)           # (64, S)
    sinT = np.sin(emb).T.astype(np.float32)           # (64, S)
    # sinrs is laid out at SOURCE row positions so that tmp[dest] =
    # st[src] * sinrs[src] has equal input base partitions (ISA rule):
    #   dest 0-31  <- src 32-63: factor -sin[dest]; stored at rows 32-63
    #   dest 32-63 <- src 0-31:  factor +sin[dest]; stored at rows 0-31
    # (sinT rows 0-31 and 32-63 are identical, so signs are what matter)
    sinrs = np.empty_like(sinT)
    sinrs[0:32] = sinT[0:32]
    sinrs[32:64] = -sinT[32:64]
    cos2 = np.tile(cosT, (2, 1))                      # (128, S) two heads/chunk
    sinr2 = np.tile(sinrs, (2, 1))
    return np.ascontiguousarray(cos2), np.ascontiguousarray(sinr2)


def _build_nc():
    nc = bacc.Bacc("TRN2", target_bir_lowering=False)
    xT = nc.dram_tensor("xT", [128, KT, S], F32R, kind="ExternalInput")
    wqT = nc.dram_tensor("wqT", [128, KT, GD], F32R, kind="ExternalInput")
    wkT = nc.dram_tensor("wkT", [128, KT, GD], F32R, kind="ExternalInput")
    wvT = nc.dram_tensor("wvT", [128, KT, GD], F32R, kind="ExternalInput")
    woT = nc.dram_tensor("woT", [128, 2, H], F32R, kind="ExternalInput")
    cos2 = nc.dram_tensor("cos2", [128, S], F32R, kind="ExternalInput")
    sinr = nc.dram_tensor("sinr", [128, S], F32R, kind="ExternalInput")
    onesd = nc.dram_tensor("onesd", [128, MC * GH], F32R, kind="ExternalInput")
    zerosd = nc.dram_tensor("zerosd", [128, S], F32R, kind="ExternalInput")
    outp = nc.dram_tensor("outp", [H, S], F32, kind="ExternalOutput")

    import os as _os
    _repeat = int(_os.environ.get('KERNEL_REPEAT', '1'))
    _mode = _os.environ.get('KERNEL_REPEAT_MODE', 'all')
    with tile.TileContext(nc) as tc:
        with (
            tc.tile_pool(name="const", bufs=1) as const,
            tc.tile_pool(name="persist", bufs=1) as persist,
        ):
            cos_sb = const.tile([128, S], F32R)
            sinr_sb = const.tile([128, S], F32R)

            qT_sb = persist.tile([128, 2, S], F32R)
            # kTz: per-head slots with the other head's 64 rows zeroed, so
            # scores matmuls can run at K=128 (K=64 fp32r costs 2.4x per row)
            kTz_sb = persist.tile([128, GH, S], F32R)
            v_sb = persist.tile([128, MC, GH, D + 1], F32R)

            for _rep in range(_repeat):
                _do_c = (_rep == 0) or (_mode in ('all', 'c'))
                if _rep == 0 or _mode in ('all', 'b'):
                    # ------------- phase B: projections + rope -------------
                    with (
                        tc.tile_pool(name="ldw", bufs=1) as ldw,
                    ):
                        xT_sb = ldw.tile([128, KT, S], F32R)
                        wqT_sb = ldw.tile([128, KT, GD], F32R)
                        wkT_sb = ldw.tile([128, KT, GD], F32R)
                        for kt in range(KT):
                            nc.sync.dma_start(xT_sb[:, kt, :], xT[:, kt, :])
                        nc.sync.dma_start(wqT_sb[:], wqT[:])
                        nc.sync.dma_start(wkT_sb[:], wkT[:])
                        nc.sync.dma_start(cos_sb[:], cos2[:])
                        nc.sync.dma_start(sinr_sb[:], sinr[:])
                        # zero halves of kTz: even heads rows 64-127, odd 0-63
                        zview = zerosd.rearrange("(a p) s -> p a s", a=2)
                        nc.sync.dma_start(kTz_sb[64:128, 0::2, :], zview)
                        nc.sync.dma_start(kTz_sb[0:64, 1::2, :], zview)
                        nc.sync.dma_start(
                            v_sb[:, :, :, D:D + 1],
                            onesd.rearrange("p (a b o) -> p a b o", a=MC, o=1),
                        )

                        # v projection in its own pool scope so wv's 1MB is
                        # freed before the rope staging pools open
                        with (
                            tc.tile_pool(name="ldw2", bufs=1) as ldw2,
                            tc.tile_pool(name="vpsum", bufs=4, space="PSUM") as vpsum,
                        ):
                            wvT_sb = ldw2.tile([128, KT, GD], F32R)
                            nc.sync.dma_start(wvT_sb[:], wvT[:])
                            for mc in range(MC):
                                ps = vpsum.tile([128, GD], F32, tag="vp")
                                for kt in range(KT):
                                    nc.tensor.matmul(
                                        ps[:], xT_sb[:, kt, bass.ts(mc, 128)],
                                        wvT_sb[:, kt, :],
                                        start=(kt == 0), stop=(kt == KT - 1),
                                    )
                                nc.vector.tensor_copy(
                                    v_sb[:, mc, :, 0:D],
                                    ps.rearrange("p (h d) -> p h d", h=GH),
                                )

                        with (
                            tc.tile_pool(name="stpool", bufs=2) as stpool,
                            tc.tile_pool(name="ptmp", bufs=1) as ptmp,
                            tc.tile_pool(name="ppsum", bufs=4, space="PSUM") as ppsum,
                        ):
                            # q/k projections + rope (ACT stages psum->sbuf;
                            # DVE/gpsimd rope at full-seq free dim)
                            for w_sb, which in ((wqT_sb, "q"), (wkT_sb, "k")):
                                for m in range(2):
                                    st = stpool.tile([128, S], F32R, tag="stage",
                                                     name=f"st_{_rep}_{which}_{m}")
                                    for nb in range(QB):
                                        sl = bass.ts(nb, 512)
                                        ps = ppsum.tile([128, 512], F32, tag="pp")
                                        for kt in range(KT):
                                            nc.tensor.matmul(
                                                ps[:], w_sb[:, kt, bass.ts(m, 128)],
                                                xT_sb[:, kt, sl],
                                                start=(kt == 0), stop=(kt == KT - 1),
                                            )
                                        nc.scalar.copy(st[:, sl], ps[:])
                                    tmpR = ptmp.tile([128, S], F32R, tag="rot",
                                                     name=f"tr_{_rep}_{which}_{m}")
                                    nc.vector.tensor_mul(tmpR[0:32], st[32:64], sinr_sb[32:64])
                                    nc.vector.tensor_mul(tmpR[32:64], st[0:32], sinr_sb[0:32])
                                    nc.gpsimd.tensor_mul(tmpR[64:96], st[96:128], sinr_sb[96:128])
                                    nc.gpsimd.tensor_mul(tmpR[96:128], st[64:96], sinr_sb[64:96])
                                    if which == "q":
                                        nc.vector.tensor_mul(qT_sb[:, m, :], st[:], cos_sb[:])
                                        nc.vector.tensor_add(qT_sb[:, m, :],
                                                             qT_sb[:, m, :], tmpR[:])
                                    else:
                                        tmpC = ptmp.tile([128, S], F32R, tag="cosp",
                                                         name=f"tc_{_rep}_{m}")
                                        nc.vector.tensor_mul(tmpC[:], st[:], cos_sb[:])
                                        nc.vector.tensor_add(
                                            kTz_sb[0:64, 2 * m, :],
                                            tmpC[0:64], tmpR[0:64])
                                        nc.vector.tensor_add(
                                            kTz_sb[64:128, 2 * m + 1, :],
                                            tmpC[64:128], tmpR[64:128])

                # ------------- phase C: attention + output projection -------------
                # Software-pipelined: iteration i computes scores+exp for item i
                # and the attn@V / normalize for item i-1.
                if not _do_c:
                    continue
                with (
                    tc.tile_pool(name="cpersist", bufs=1) as cpersist,
                    tc.tile_pool(name="es", bufs=2) as es_pool,
                    tc.tile_pool(name="atmp", bufs=4) as atmp,
                    tc.tile_pool(name="osb", bufs=4) as osb_pool,
                    tc.tile_pool(name="spsum", bufs=2, space="PSUM") as spsum,
                    tc.tile_pool(name="smallps", bufs=2, space="PSUM") as smallps,
                ):
                    aoT_sb = cpersist.tile([128, 2, S], F32R)
                    woT_sb = cpersist.tile([128, 2, H], F32R)
                    nc.sync.dma_start(woT_sb[:], woT[:])

                    def scores_exp(qb, h, i):
                        qsl = bass.ts(qb, 512)
                        # two half-item es tiles: ao(i) releases the first half
                        # early so exp(i+2) can start before ao(i) finishes
                        esA = es_pool.tile([128, MC // 2, 512], F32R, tag="esa",
                                           name=f"esa_{_rep}_{i}")
                        esB = es_pool.tile([128, MC // 2, 512], F32R, tag="esb",
                                           name=f"esb_{_rep}_{i}")
                        kc0 = 0
                        for gsz in (3, 3, 2, 3, 3, 2):
                            es = esA if kc0 < MC // 2 else esB
                            off = 0 if kc0 < MC // 2 else MC // 2
                            sp = spsum.tile([128, 3, 512], F32, tag="sp",
                                            name=f"sp_{_rep}_{i}_{kc0}")
                            for j in range(gsz):
                                kc = kc0 + j
                                nc.tensor.matmul(
                                    sp[:, j, :],
                                    kTz_sb[:, h, bass.ts(kc, 128)],
                                    qT_sb[:, h // 2, qsl],
                                    start=True, stop=True,
                                )
                            nc.scalar.activation(
                                es[:, kc0 - off:kc0 - off + gsz, :],
                                sp[:, 0:gsz, :],
                                EXP, scale=SCALE,
                            )
                            kc0 += gsz
                        return (esA, esB)

                    def attn_out(qb, h, es, i):
                        esA, esB = es
                        qsl = bass.ts(qb, 512)
                        hc, hr = h // 2, (h % 2) * 64
                        ao = smallps.tile([D + 1, 512], F32, tag="ao",
                                          name=f"ao_{_rep}_{i}")
                        for kc in range(MC):
                            eshalf = esA if kc < MC // 2 else esB
                            nc.tensor.matmul(
                                ao[:], v_sb[:, kc, h, :],
                                eshalf[:, kc % (MC // 2), :],
                                start=(kc == 0), stop=(kc == MC - 1),
                            )
                        rcp = atmp.tile([1, 512], F32R, tag="rcp")
                        with nc.allow_low_precision(reason="f32r is fp32-width"):
                            nc.vector.reciprocal(rcp[:], ao[D:D + 1, :])
                        bsb = atmp.tile([D, 512], F32R, tag="bsb")
                        nc.gpsimd.partition_broadcast(bsb[:], rcp[:])
                        nc.vector.tensor_mul(
                            aoT_sb[hr:hr + 64, hc, qsl], ao[0:D, :], bsb[:],
                        )

                    def oproj(qb):
                        # transposed output: partial^T[hid, seq]
                        qsl = bass.ts(qb, 512)
                        for hc8 in range(8):
                            ps = smallps.tile([128, 512], F32, tag="ao",
                                              name=f"op_{_rep}_{qb}_{hc8}")
                            for kt in range(2):
                                nc.tensor.matmul(
                                    ps[:], woT_sb[:, kt, bass.ts(hc8, 128)],
                                    aoT_sb[:, kt, qsl],
                                    start=(kt == 0), stop=(kt == 1),
                                )
                            o_sb = osb_pool.tile([128, 512], F32, tag="ot")
                            nc.vector.tensor_copy(o_sb[:], ps[:])
                            nc.sync.dma_start(
                                outp[bass.ts(hc8, 128), qsl], o_sb[:],
                            )

                    items = [(qb, h) for qb in range(QB) for h in range(GH)]
                    pending = None
                    for i, (qb, h) in enumerate(items):
                        es = scores_exp(qb, h, i)
                        if pending is not None:
                            pqb, ph, pes, pi = pending
                            attn_out(pqb, ph, pes, pi)
                            if ph == GH - 1:
                                oproj(pqb)
                        pending = (qb, h, es, i)
                    pqb, ph, pes, pi = pending
                    attn_out(pqb, ph, pes, pi)
                    oproj(pqb)

    nc.compile()
    return nc


_NC_CACHE = None
_last_in_maps = None


def _get_nc():
    global _NC_CACHE
    if _NC_CACHE is None:
        _NC_CACHE = _build_nc()
    return _NC_CACHE


def make_in_maps(x, Wq, Wk, Wv, Wo):
    cos2, sinr = _rope_tables()

    def fold(a):  # [X, F] with X=128*KTI -> [128, KTI, F]
        kti = a.shape[0] // 128
        return np.ascontiguousarray(a.reshape(kti, 128, -1).transpose(1, 0, 2))

    in_maps = []
    for core in range(8):
        b, g = core // 4, core % 4
        rows = slice(g * GD, (g + 1) * GD)
        in_maps.append({
            "xT": fold(np.ascontiguousarray(x[b].T)),          # (1024, S)
            "wqT": fold(np.ascontiguousarray(Wq[rows].T)),     # (1024, 256)
            "wkT": fold(np.ascontiguousarray(Wk[rows].T)),
            "wvT": fold(np.ascontiguousarray(Wv[rows].T)),
            "woT": fold(np.ascontiguousarray(Wo[:, rows].T)),  # (256, 1024)
            "cos2": cos2,
            "sinr": sinr,
            "onesd": np.ones((128, MC * GH), dtype=np.float32),
            "zerosd": np.zeros((128, S), dtype=np.float32),
        })
    return in_maps


def kernel(x, Wq, Wk, Wv, Wo):
    x = np.asarray(x, dtype=np.float32)
    Wq = np.asarray(Wq, dtype=np.float32)
    Wk = np.asarray(Wk, dtype=np.float32)
    Wv = np.asarray(Wv, dtype=np.float32)
    Wo = np.asarray(Wo, dtype=np.float32)

    global _last_in_maps
    in_maps = make_in_maps(x, Wq, Wk, Wv, Wo)
    _last_in_maps = in_maps
    nc = _get_nc()
    res = run_bass_kernel_spmd(nc, in_maps, core_ids=list(range(8)))
    out = np.zeros((B, S, H), dtype=np.float32)
    for core in range(8):
        out[core // 4] += res.results[core]["outp"].T
    return out

